# revision 1
# baseline (speedup 1.0000x reference)
"""Trainium2 Bass kernel: batched attention-distribution forward.

Computes, for x:[B,S,F], Wq/Wk:[F,D], bq/bk:[D]:
    q = x@Wq + bq ; k = x@Wk + bk
    qkt = q @ k^T                    # [B,S,S]
    dist = softmax(qkt / rowmax(qkt))

Sharding: 8 NeuronCores, core c -> batch c//2, query-row half c%2.
Each core emits a [2048, 4096] f32 slab (33.5 MB) -> memory-bound on the
HBM write (~358 GB/s/core).

Per-core pipeline, per 128-row tile. PSUM is one [128,4096] tensor
(all 8 banks); Tile tracks dependencies at bank granularity, so the next
tile's matmuls stream into each 512-column bank as soon as this tile's
exp over that range has consumed it:
  PE    : 8x N=512 matmuls (bf16 in, f32 PSUM out)
  DVE   : 2x reduce_max halves -> combine -> 1/M  (rowmax > 0 here)
  ACT   : 2x Exp(scale=1/M, bias=-1) PSUM->SBUF bf16 e, accum_out=sums
          (softmax is shift invariant: exp(z-1)/sum == reference)
  DVE   : 1/sum; normalize left span bf16->f32 (2x two-port mode)
  ACT   : normalize right span bf16->f32
  DMA   : both spans via HWDGE (an in-place-4x + SWDGE cast-DMA variant
          was ~20% faster but intermittently raced on silicon)

Host-side prep is layout only (transpose x to [F,S], append a ones-row so
the bias rides inside the matmul contraction, pre-round to bf16); every
FLOP runs on device.
"""

from contextlib import ExitStack

import ml_dtypes
import numpy as np

import concourse.bacc as bacc
import concourse.bass as bass
import concourse.mybir as mybir
import concourse.tile as tile
from concourse.bass_utils import run_bass_kernel_spmd

B, S, F, D = 4, 4096, 33, 64
NCORES = 8
HALF = S // 2        # query rows per core
PT = 128             # rows per tile
NT = HALF // PT      # 16 tiles
FA = F + 1           # features + ones-row (bias folded into matmul)
XSPLIT = 2560        # normalize: [0:XSPLIT] on DVE, rest on ACT

F32 = mybir.dt.float32
BF16 = mybir.dt.bfloat16


def build_bass(repeat: int = 1) -> bass.Bass:
    nc = bacc.Bacc(trn_type="TRN2")
    # Packed inputs: one DMA per tensor.
    # xaw = [x[b]^T aug | Wk aug] ; xqw = [x[b]^T aug (this half) | Wq aug]
    xaw = nc.declare_dram_parameter("xaw", [FA, S + D], BF16, isOutput=False)
    xqw = nc.declare_dram_parameter("xqw", [FA, HALF + D], BF16, isOutput=False)
    out = nc.declare_dram_parameter("out", [HALF, S], F32, isOutput=True)

    Exp = mybir.ActivationFunctionType.Exp

    with tile.TileContext(nc) as tc, ExitStack() as ctx:
        singles = ctx.enter_context(tc.tile_pool(name="singles", bufs=1))
        psum = ctx.enter_context(tc.tile_pool(name="psum", bufs=1, space="PSUM"))
        e_pool = ctx.enter_context(tc.tile_pool(name="e", bufs=3))
        e32_pool = ctx.enter_context(tc.tile_pool(name="e32", bufs=3))
        e32L_pool = ctx.enter_context(tc.tile_pool(name="e32L", bufs=2))
        stats = ctx.enter_context(tc.tile_pool(name="stats", bufs=8))

        # ---- load inputs ----
        xaw_sb = singles.tile([FA, S + D], BF16)
        nc.sync.dma_start(out=xaw_sb[:, :], in_=xaw[:, :])
        xqw_sb = singles.tile([FA, HALF + D], BF16)
        nc.sync.dma_start(out=xqw_sb[:, :], in_=xqw[:, :])
        neg1 = singles.tile([PT, 1], F32)
        nc.vector.memset(neg1[:, :], -1.0)

        # one tensor spanning all of PSUM; sliced at bank granularity
        big = psum.tile([PT, S], F32)

        # ---- projections: qT = (xq^T @ Wq)^T, kT likewise (bf16) ----
        qT = singles.tile([D, HALF], BF16)
        kT = singles.tile([D, S], BF16)

        # qT first half first (tiles 0-7 need it), then kT (tile 0 needs all
        # of it), then qT second half. PSUM ranges rotate; copies alternate
        # DVE/ACT so the prologue isn't serialized on one engine.
        def proj(psum_c0, lhsT, rhs_sb, rhs_c0, dst, dst_c0, eng):
            for j in range(2):
                nc.tensor.matmul(
                    big[0:D, psum_c0 + j * 512:psum_c0 + (j + 1) * 512],
                    lhsT=lhsT,
                    rhs=rhs_sb[:, rhs_c0 + j * 512:rhs_c0 + (j + 1) * 512],
                    start=True, stop=True,
                )
            src = big[0:D, psum_c0:psum_c0 + 1024]
            if eng == "v":
                nc.vector.tensor_copy(dst[:, dst_c0:dst_c0 + 1024], src)
            else:
                nc.scalar.copy(dst[:, dst_c0:dst_c0 + 1024], src)

        wq_l = xqw_sb[:, HALF:HALF + D]
        wk_l = xaw_sb[:, S:S + D]
        # Only what pass-A(tile 0, chunk 0) needs runs up front; the other
        # projections interleave into step 0 so the pipeline starts ~5us
        # earlier. Timing builds (repeat > 1) keep the full up-front
        # prologue: re-projecting inside the For_i would overwrite kT while
        # the previous repetition's pass-B still reads it.
        proj(3072, wq_l, xqw_sb, 0, qT, 0, "v")       # qT half 0
        proj(2048, wk_l, xaw_sb, 0, kT, 0, "s")       # kT chunk 0
        if repeat > 1:
            proj(1024, wk_l, xaw_sb, 1024, kT, 1024, "v")
            proj(0, wk_l, xaw_sb, 2048, kT, 2048, "s")
            proj(1024, wk_l, xaw_sb, 3072, kT, 3072, "v")
            proj(0, wq_l, xqw_sb, 1024, qT, 1024, "s")

        # ---- main loop: software-pipelined two-pass softmax ----
        # Pass A (tile u = step, LOOKAHEAD tiles ahead): qkt chunk -> row
        # max, qkt discarded. Pass B (tile v = step-LOOKAHEAD): recompute
        # qkt, exp immediately with the already-known 1/M, normalize, DMA.
        # PE work doubles (cheap), but the "all maxes before any exp" join
        # leaves the steady-state recurrence: each PSUM bank range hosts an
        # independent exp(v-1) -> A-mm(u) -> A-max -> B-mm(v) -> exp(v)
        # chain, staggered across the four 1024-col ranges.
        LOOKAHEAD = 2
        rep_ctx = tc.For_i(0, repeat, 1) if repeat > 1 else None
        if rep_ctx is not None:
            ctx.enter_context(rep_ctx)
        rM_of = {}
        for step in range(NT + LOOKAHEAD):
            u = step
            v = step - LOOKAHEAD
            if u < NT:
                lhsT = qT[:, u * PT:(u + 1) * PT]
                mvec = stats.tile([PT, 4], F32, tag="mvec")
                for c in range(4):
                    if step == 0 and repeat == 1 and c >= 1:
                        # stream the remaining kT projections in just before
                        # the first tile's chunk that needs them, using PSUM
                        # ranges this step has already drained
                        pr = {1: 3072, 2: 2048, 3: 0}[c]
                        eng = {1: "v", 2: "s", 3: "v"}[c]
                        proj(pr, wk_l, xaw_sb, c * 1024, kT, c * 1024, eng)
                    for j in range(2):
                        c0 = c * 1024 + j * 512
                        nc.tensor.matmul(
                            big[:, c0:c0 + 512],
                            lhsT=lhsT,
                            rhs=kT[:, c0:c0 + 512],
                            start=True, stop=True,
                        )
                    nc.vector.reduce_max(
                        mvec[:, c:c + 1], big[:, c * 1024:(c + 1) * 1024],
                        axis=mybir.AxisListType.X,
                    )
                if step == 0 and repeat == 1:
                    proj(1024, wq_l, xqw_sb, 1024, qT, 1024, "s")
                with tc.high_priority(offset=24):
                    m = stats.tile([PT, 1], F32, tag="m")
                    nc.vector.reduce_max(
                        m[:, 0:1], mvec[:, :], axis=mybir.AxisListType.X
                    )
                    rM = stats.tile([PT, 1], F32, tag="rM")
                    nc.vector.reciprocal(rM[:, 0:1], m[:, 0:1])
                rM_of[u] = rM

            if v < 0:
                continue
            lhsT = qT[:, v * PT:(v + 1) * PT]
            rM = rM_of.pop(v)
            e = e_pool.tile([PT, S], BF16)
            svec = stats.tile([PT, 4], F32, tag="svec")
            for c in range(4):
                for j in range(2):
                    c0 = c * 1024 + j * 512
                    nc.tensor.matmul(
                        big[:, c0:c0 + 512],
                        lhsT=lhsT,
                        rhs=kT[:, c0:c0 + 512],
                        start=True, stop=True,
                    )
                nc.scalar.activation(
                    out=e[:, c * 1024:(c + 1) * 1024],
                    in_=big[:, c * 1024:(c + 1) * 1024],
                    func=Exp,
                    bias=neg1[:, 0:1],
                    scale=rM[:, 0:1],
                    accum_out=svec[:, c:c + 1],
                )

            # post-exp chain unblocks this tile's DMAs.
            with tc.high_priority(offset=24):
                ssum = stats.tile([PT, 1], F32, tag="ssum")
                nc.vector.reduce_sum(
                    ssum[:, 0:1], svec[:, :], axis=mybir.AxisListType.X
                )
                rs = stats.tile([PT, 1], F32, tag="rs")
                nc.vector.reciprocal(rs[:, 0:1], ssum[:, 0:1])

                # left span: normalize bf16 -> f32 on DVE (2x two-port
                # mode, fresh destination), plain HWDGE DMA. The in-place
                # 4x normalize + SWDGE cast-DMA variant was ~20% faster but
                # produced intermittent garbage on silicon (suspect Q7
                # descriptor-ring vs DVE two-port SBUF lockout); this path
                # never flaked.
                eL = e32L_pool.tile([PT, XSPLIT], F32)
                nc.vector.tensor_scalar_mul(
                    eL[:, :], e[:, 0:XSPLIT], rs[:, 0:1]
                )
                nc.sync.dma_start(
                    out=out[v * PT:(v + 1) * PT, 0:XSPLIT],
                    in_=eL[:, :],
                )
                # right span: normalize bf16 -> f32 on ACT, plain DMA
                e32 = e32_pool.tile([PT, S - XSPLIT], F32)
                nc.scalar.mul(e32[:, :], e[:, XSPLIT:S], rs[:, 0:1])
                nc.sync.dma_start(
                    out=out[v * PT:(v + 1) * PT, XSPLIT:S], in_=e32[:, :]
                )

    nc.compile()
    return nc


_NC = None


def _get_nc() -> bass.Bass:
    global _NC
    if _NC is None:
        _NC = build_bass()
    return _NC


_NC_TIMED = {}


def _get_nc_timed(repeat: int) -> bass.Bass:
    if repeat not in _NC_TIMED:
        _NC_TIMED[repeat] = build_bass(repeat)
    return _NC_TIMED[repeat]


def prepare_in_maps(inputs: dict) -> list[dict]:
    x = np.ascontiguousarray(np.asarray(inputs["x"], dtype=np.float32))
    Wq = np.asarray(inputs["Wq"], dtype=np.float32)
    bq = np.asarray(inputs["bq"], dtype=np.float32)
    Wk = np.asarray(inputs["Wk"], dtype=np.float32)
    bk = np.asarray(inputs["bk"], dtype=np.float32)

    wq_aug = np.concatenate([Wq, bq[None, :]], axis=0)
    wk_aug = np.concatenate([Wk, bk[None, :]], axis=0)

    in_maps = []
    xaw_cache = {}
    for c in range(NCORES):
        b, h = c // 2, c % 2
        if b not in xaw_cache:
            xaw = np.empty((FA, S + D), ml_dtypes.bfloat16)
            xaw[:F, :S] = x[b].T
            xaw[F, :S] = 1.0
            xaw[:, S:] = wk_aug
            xaw_cache[b] = xaw
        xaw = xaw_cache[b]
        xqw = np.empty((FA, HALF + D), ml_dtypes.bfloat16)
        xqw[:, :HALF] = xaw[:, h * HALF:(h + 1) * HALF]
        xqw[:, HALF:] = wq_aug
        in_maps.append({"xaw": xaw, "xqw": xqw})
    return in_maps


def run(in_maps: list[dict], **kwargs):
    return run_bass_kernel_spmd(
        _get_nc(), in_maps, core_ids=list(range(NCORES)), **kwargs
    )


def assemble(results: list[dict]) -> np.ndarray:
    out = np.empty((B, S, S), np.float32)
    for c in range(NCORES):
        b, h = c // 2, c % 2
        out[b, h * HALF:(h + 1) * HALF, :] = results[c]["out"]
    return out


def kernel(**inputs) -> np.ndarray:
    res = run(prepare_in_maps(inputs))
    return assemble(res.results)

